# revision 18
# baseline (speedup 1.0000x reference)
"""Distributed GQA attention layer (seq=2048, dim=4096, 32 q heads / 8 kv heads,
rope theta=5e5, causal) on 8 TRN2 NeuronCores.

Sharding: tensor-parallel over heads. Core c owns q heads 4c..4c+3 and kv head c.
Each core computes its 4 heads' attention output in transposed layout
attnT_local [512, 2048], an AllGather over the partition axis assembles
attnT_full [4096, 2048], and each core then computes 512 output columns of the
final projection: out_c [2048, 512] = attnT_full.T @ woT_c.  The host
concatenates the 8 column blocks.

Device-side layout tricks (all host-prepped):
 - x fed transposed [dim, seq]; all weights fed as [dim(contract), out].
 - rope pairs (2i, 2i+1) are permuted to half-split form by permuting wq/wk
   rows, making rope a "rotate-half": r = t*CH + swap_halves(t)*SH, where
   swap_halves is a 128x128 permutation matmul and CH/SH are [128, seq]
   tables. The q·k inner product is invariant under the shared permutation.
 - 1/sqrt(hd) folded into wq.
 - softmax runs unnormalized; a ones-column appended to v makes the PV matmul
   accumulate the denominator in psum column 128, and the normalization is
   fused into the psum->sbuf copy as a per-partition activation scale.
"""
import sys

sys.path.insert(0, "/opt/trn_rl_repo")
import numpy as np

import concourse.bass as bass
import concourse.mybir as mybir
import concourse.tile as tile
from concourse import bacc
from concourse.bass_utils import run_bass_kernel_spmd

import os as _os
SEQ = int(_os.environ.get("KERN_SEQ", "2048"))
DIM = int(_os.environ.get("KERN_DIM", "4096"))
NH, NKV, HD = 32, 8, 128
THETA = 500000.0
NCORE = 8
HPC = NH // NCORE          # 4 q heads per core
HALF = HD // 2
SCALE = 1.0 / np.sqrt(HD)
NDT = DIM // 128           # contraction tiles for qkv projections
NET = (NH * HD) // 128     # e-dim tiles for the output projection (32)
NCH = SEQ // 512           # 4 seq chunks
F32 = mybir.dt.float32
AF = mybir.ActivationFunctionType
ALU = mybir.AluOpType

import os
COMPUTE = os.environ.get("KERN_COMPUTE", "bf16")  # f32 | f32r | bf16
if COMPUTE == "bf16":
    CD = mybir.dt.bfloat16
else:
    CD = F32


def _mm(ap):
    """Matmul-operand view: reinterpret f32 tiles as float32r when requested."""
    if COMPUTE == "f32r":
        return ap.bitcast(mybir.dt.float32r)
    return ap


def build():
    nc = bacc.Bacc("TRN2", target_bir_lowering=False, debug=False, num_devices=NCORE)
    x_e = nc.dram_tensor("x", [DIM, SEQ], CD, kind="ExternalInput")
    wq_e = nc.dram_tensor("wq", [DIM, HPC * HD], CD, kind="ExternalInput")
    wk_e = nc.dram_tensor("wk", [DIM, HD], CD, kind="ExternalInput")
    wv_e = nc.dram_tensor("wv", [DIM, HD], CD, kind="ExternalInput")
    wo_e = nc.dram_tensor("wo", [NH * HD, HPC * HD], CD, kind="ExternalInput")
    ch_e = nc.dram_tensor("cosz", [HD, SEQ], CD, kind="ExternalInput")
    sh_e = nc.dram_tensor("sinz", [HD, SEQ], CD, kind="ExternalInput")
    sw_e = nc.dram_tensor("swp", [HD, HD], CD, kind="ExternalInput")
    id_e = nc.dram_tensor("iden", [HD, HD], CD, kind="ExternalInput")
    mk_e = nc.dram_tensor("mask", [HD, HD], F32, kind="ExternalInput")
    out_e = nc.dram_tensor("out", [SEQ, HPC * HD], F32, kind="ExternalOutput")

    agin = [nc.dram_tensor(f"agin{j}", [HPC * HD, 512], CD) for j in range(NCH)]
    agout = [
        nc.dram_tensor(f"agout{j}", [NH * HD, 512], CD, addr_space="Shared")
        for j in range(NCH)
    ]

    with tile.TileContext(nc) as tc:
        _build_body(nc, tc, locals())
    nc.compile()
    return nc


def _build_body(nc, tc, ext):
    from contextlib import ExitStack

    x_e, wq_e, wk_e, wv_e, wo_e = (ext[k] for k in ("x_e", "wq_e", "wk_e", "wv_e", "wo_e"))
    ch_e, sh_e, sw_e, id_e, mk_e = (ext[k] for k in ("ch_e", "sh_e", "sw_e", "id_e", "mk_e"))
    out_e, agin, agout = ext["out_e"], ext["agin"], ext["agout"]

    with ExitStack() as ctx:
        consts = ctx.enter_context(tc.tile_pool(name="consts", bufs=1))
        qkv = ctx.enter_context(tc.tile_pool(name="qkv", bufs=1))
        rope = ctx.enter_context(tc.tile_pool(name="rope", bufs=2))
        epool = ctx.enter_context(tc.tile_pool(name="epool", bufs=3))
        atp = ctx.enter_context(tc.tile_pool(name="atp", bufs=2))
        small = ctx.enter_context(tc.tile_pool(name="small", bufs=2))
        ocp = ctx.enter_context(tc.tile_pool(name="ocp", bufs=2))
        ps512 = ctx.enter_context(tc.tile_pool(name="ps512", bufs=2, space="PSUM"))
        psbank = ctx.enter_context(tc.tile_pool(name="psbank", bufs=4, space="PSUM"))
        pswo = ctx.enter_context(tc.tile_pool(name="pswo", bufs=1, space="PSUM"))
        pstr = ctx.enter_context(tc.tile_pool(name="pstr", bufs=1, space="PSUM"))

        # ---- constants / persistent tensors ----
        ch_sb = consts.tile([HD, SEQ], CD, name="ch_sb")
        sh_sb = consts.tile([HD, SEQ], CD, name="sh_sb")
        sw_sb = consts.tile([HD, HD], CD, name="sw_sb")
        nc.sync.dma_start(sw_sb[:], sw_e[:, :])
        id_sb = consts.tile([HD, HD], CD, name="id_sb")
        nc.sync.dma_start(id_sb[:], id_e[:, :])
        mk_sb = consts.tile([HD, HD], F32, name="mk_sb")
        nc.sync.dma_start(mk_sb[:], mk_e[:, :])
        # resident weights: wq [d-tile, m] blocks, wk/wv per d-tile, wo per e-tile
        # (loads are emitted inside P1, interleaved with chunk-0 x tiles, so the
        # first projection matmul starts after a handful of DMAs)
        wq_sb = consts.tile([128, NDT * HPC * 128], CD, name="wq_sb")
        wk_sb = consts.tile([128, NDT * 128], CD, name="wk_sb")
        wv_sb = consts.tile([128, NDT * 128], CD, name="wv_sb")
        wo_sb = consts.tile([128, NET * 512], CD, name="wo_sb")
        qt_sb = qkv.tile([128, HPC * SEQ], CD, name="qt_sb")      # [hd, (h, seq)]
        kt_sb = qkv.tile([128, SEQ], CD, name="kt_sb")            # [hd, seq]
        v_sb = qkv.tile([128, (SEQ // 128) * (HD + 1)], CD, name="v_sb")  # [seqP,(t,129)]
        # ones column of v-hat (col 128 of each 129-block)
        nc.vector.memset(
            v_sb[:].rearrange("p (t c) -> p t c", c=HD + 1)[:, :, HD:HD + 1], 1.0
        )

        # ======== interleaved emission: proj j -> attn j (+ woven wo j-1) ========
        with tc.tile_pool(name="xin", bufs=34) as xin, tc.tile_pool(name="agp", bufs=12) as agp:

            def emit_proj(j):
                xts = []
                for d in range(NDT):
                    xt = xin.tile([128, 512], CD, tag="xin", name=f"x_{j}_{d}")
                    nc.sync.dma_start(xt[:], x_e[128 * d:128 * (d + 1), 512 * j:512 * (j + 1)])
                    xts.append(xt)
                    if j == 0:
                        nc.sync.dma_start(
                            wq_sb[:, 512 * d:512 * (d + 1)], wq_e[128 * d:128 * (d + 1), :]
                        )
                        nc.sync.dma_start(
                            wk_sb[:, 128 * d:128 * (d + 1)], wk_e[128 * d:128 * (d + 1), :]
                        )
                        nc.sync.dma_start(
                            wv_sb[:, 128 * d:128 * (d + 1)], wv_e[128 * d:128 * (d + 1), :]
                        )
                if j == 0:
                    nc.sync.dma_start(ch_sb[:], ch_e[:, :])
                    nc.sync.dma_start(sh_sb[:], sh_e[:, :])
                if j == min(1, NCH - 1):
                    for et in range(NET):
                        nc.sync.dma_start(
                            wo_sb[:, 512 * et:512 * (et + 1)],
                            wo_e[128 * et:128 * (et + 1), :],
                        )
                # q heads (m=0..3) and k (m=4): outputs in [hd, seq] layout
                for m in range(HPC + 1):
                    acc = ps512.tile([128, 512], F32, tag="b512", name=f"acc_{j}_{m}")
                    for d in range(NDT):
                        wsl = (
                            wq_sb[:, 512 * d + 128 * m: 512 * d + 128 * (m + 1)]
                            if m < HPC
                            else wk_sb[:, 128 * d:128 * (d + 1)]
                        )
                        nc.tensor.matmul(
                            acc[:], _mm(wsl), _mm(xts[d][:]), start=(d == 0), stop=(d == NDT - 1)
                        )
                    # rope: dest = acc*CH + (swap @ acc)*SH
                    t_sb = rope.tile([128, 512], CD, tag="tsb", name=f"t_{j}_{m}")
                    nc.scalar.activation(t_sb[:], acc[:], AF.Copy)
                    ups = ps512.tile([128, 512], F32, tag="b512", name=f"u_{j}_{m}")
                    nc.tensor.matmul(ups[:], _mm(sw_sb[:]), _mm(t_sb[:]), start=True, stop=True)
                    m1 = rope.tile([128, 512], CD, tag="m1", name=f"m1_{j}_{m}")
                    nc.vector.tensor_tensor(
                        m1[:], t_sb[:], ch_sb[:, 512 * j:512 * (j + 1)], op=ALU.mult
                    )
                    if m < HPC:
                        dest = qt_sb[:, SEQ * m + 512 * j: SEQ * m + 512 * (j + 1)]
                    else:
                        dest = kt_sb[:, 512 * j:512 * (j + 1)]
                    nc.vector.tensor_tensor(
                        dest, ups[:], sh_sb[:, 512 * j:512 * (j + 1)], op=ALU.mult
                    )
                    nc.vector.tensor_add(dest, dest, m1[:])
                # v: wide-N [hd, seq] projection, then PE-transpose to natural layout
                vt_ps = ps512.tile([128, 512], F32, tag="b512", name=f"vt_{j}")
                for d in range(NDT):
                    nc.tensor.matmul(
                        vt_ps[:],
                        _mm(wv_sb[:, 128 * d:128 * (d + 1)]),
                        _mm(xts[d][:]),
                        start=(d == 0),
                        stop=(d == NDT - 1),
                    )
                vt_sb = rope.tile([128, 512], CD, tag="vtsb", name=f"vt_sb_{j}")
                nc.scalar.activation(vt_sb[:], vt_ps[:], AF.Copy)
                for st in range(4):
                    t = 4 * j + st
                    trv = pstr.tile([128, 128], CD, tag="tr", name=f"trv_{j}_{st}")
                    nc.tensor.transpose(trv[:], vt_sb[:, 128 * st:128 * (st + 1)], id_sb[:])
                    nc.vector.tensor_copy(
                        v_sb[:, (HD + 1) * t:(HD + 1) * t + HD], trv[:]
                    )

            def emit_attn_head(j, h, at_sb):
                qsl = qt_sb[:, SEQ * h + 512 * j: SEQ * h + 512 * (j + 1)]
                aps = [
                    psbank.tile([128, HD + 1], F32, tag="bank", name=f"ap_{j}_{h}_{sq}")
                    for sq in range(4)
                ]
                for skt in range(4 * j + 4):
                    r = skt - 4 * j
                    lo = 128 * r if r > 0 else 0
                    stp = ps512.tile([128, 512], F32, tag="b512", name=f"st_{j}_{h}_{skt}")
                    nc.tensor.matmul(
                        stp[:, lo:512],
                        _mm(kt_sb[:, 128 * skt:128 * (skt + 1)]),
                        _mm(qsl[:, lo:512]),
                        start=True, stop=True,
                    )
                    E = epool.tile([128, 512], CD, tag="E", name=f"E_{j}_{h}_{skt}")
                    if r >= 0:
                        mt = small.tile([128, 128], F32, tag="mt", name=f"mt_{j}_{h}_{skt}")
                        nc.vector.tensor_add(mt[:], stp[:, 128 * r:128 * (r + 1)], mk_sb[:])
                        nc.scalar.activation(E[:, 128 * r:128 * (r + 1)], mt[:], AF.Exp)
                        if r < 3:
                            nc.scalar.activation(
                                E[:, 128 * (r + 1):512], stp[:, 128 * (r + 1):512], AF.Exp
                            )
                    else:
                        nc.scalar.activation(E[:], stp[:], AF.Exp)
                    for sq in range(max(0, r), 4):
                        nc.tensor.matmul(
                            aps[sq][:],
                            _mm(E[:, 128 * sq:128 * (sq + 1)]),
                            _mm(v_sb[:, (HD + 1) * skt:(HD + 1) * (skt + 1)]),
                            start=(skt == 0),
                            stop=(skt == 4 * j + sq),
                        )
                        if skt == 4 * j + sq:  # this sq-subtile is complete
                            inv = small.tile([128, 1], F32, tag="inv", name=f"i_{j}_{h}_{sq}")
                            nc.vector.reciprocal(inv[:], aps[sq][:, HD:HD + 1])
                            an = small.tile([128, 128], CD, tag="an", name=f"an_{j}_{h}_{sq}")
                            nc.vector.tensor_scalar_mul(an[:], aps[sq][:, 0:HD], inv[:])
                            trp = pstr.tile([128, 128], CD, tag="tr", name=f"tr_{j}_{h}_{sq}")
                            nc.tensor.transpose(trp[:], an[:], id_sb[:])
                            nc.vector.tensor_copy(
                                at_sb[:, 512 * h + 128 * sq: 512 * h + 128 * (sq + 1)],
                                trp[:],
                            )

            def emit_wo_sq(jw, sq):
                # one output sub-tile [128 seq, 512 dcols]: 32 dense MMs, woven
                # between attention heads so the PE never sits in a long dense run
                wop_ps = pswo.tile([128, 512], F32, tag="wops", name=f"wop_{jw}_{sq}")
                for et in range(NET):
                    agt = agp.tile([128, 128], CD, tag="agt", name=f"ag_{jw}_{sq}_{et}")
                    nc.sync.dma_start(
                        agt[:],
                        agout[jw][128 * et:128 * (et + 1), 128 * sq:128 * (sq + 1)],
                    )
                    nc.tensor.matmul(
                        wop_ps[:],
                        _mm(agt[:]),
                        _mm(wo_sb[:, 512 * et:512 * (et + 1)]),
                        start=(et == 0),
                        stop=(et == NET - 1),
                    )
                oc = ocp.tile([128, 512], F32, tag="oc", name=f"oc_{jw}_{sq}")
                nc.scalar.activation(oc[:], wop_ps[:], AF.Copy)
                nc.sync.dma_start(
                    out_e[512 * jw + 128 * sq: 512 * jw + 128 * (sq + 1), :], oc[:]
                )

            for j in range(NCH):
                emit_proj(j)
                at_sb = atp.tile([128, HPC * 512], CD, tag="atT", name=f"atT{j}")
                for h in range(HPC):
                    emit_attn_head(j, h, at_sb)
                    if j > 0:
                        emit_wo_sq(j - 1, h)
                # ---- AllGather chunk j ----
                nc.sync.dma_start(
                    agin[j][:, :].rearrange("(h p) s -> p h s", h=HPC),
                    at_sb[:].rearrange("p (h s) -> p h s", h=HPC),
                )
                nc.gpsimd.collective_compute(
                    "AllGather",
                    ALU.bypass,
                    replica_groups=[list(range(NCORE))],
                    ins=[agin[j][:, :]],
                    outs=[agout[j][:, :]],
                )
            for sq in range(4):
                emit_wo_sq(NCH - 1, sq)


# ---------------- host side ----------------
_PERM = np.concatenate([np.arange(0, HD, 2), np.arange(1, HD, 2)])
_NC_CACHE = {}


def _get_nc():
    if "nc" not in _NC_CACHE:
        _NC_CACHE["nc"] = build()
    return _NC_CACHE["nc"]


def _prep_consts():
    freqs = 1.0 / (THETA ** (np.arange(HALF, dtype=np.float64) / HALF))
    ang = np.arange(SEQ, dtype=np.float64)[:, None] * freqs[None, :]
    cos = np.cos(ang).astype(np.float32)
    sin = np.sin(ang).astype(np.float32)
    CH = np.ascontiguousarray(np.concatenate([cos, cos], axis=1).T)
    SH = np.ascontiguousarray(np.concatenate([-sin, sin], axis=1).T)
    S_l = np.zeros((HD, HD), np.float32)
    for i in range(HD):
        S_l[(i + 64) % HD, i] = 1.0
    iden = np.eye(HD, dtype=np.float32)
    mask = np.where(
        np.arange(HD)[:, None] <= np.arange(HD)[None, :], 0.0, -1e30
    ).astype(np.float32)
    return CH, SH, S_l, iden, mask


def _cd(a):
    if COMPUTE == "bf16":
        import ml_dtypes
        return np.ascontiguousarray(a).astype(ml_dtypes.bfloat16)
    return np.ascontiguousarray(a, dtype=np.float32)


def kernel(x, wq, wk, wv, wo):
    x, wq, wk, wv, wo = (np.asarray(a, dtype=np.float32) for a in (x, wq, wk, wv, wo))
    nc = _get_nc()
    CH, SH, S_l, iden, mask = _prep_consts()
    xT = np.ascontiguousarray(x.T)
    wq_p = wq.reshape(NH, HD, DIM)[:, _PERM, :] * SCALE
    wk_p = wk.reshape(NKV, HD, DIM)[:, _PERM, :]
    xT_c = _cd(xT)
    CH_c, SH_c, S_c, id_c = _cd(CH), _cd(SH), _cd(S_l), _cd(iden)
    in_maps = []
    for c in range(NCORE):
        in_maps.append(
            {
                "x": xT_c,
                "wq": _cd(wq_p[HPC * c: HPC * (c + 1)].reshape(HPC * HD, DIM).T),
                "wk": _cd(wk_p[c].T),
                "wv": _cd(wv[HD * c: HD * (c + 1), :].T),
                "wo": _cd(wo[HPC * HD * c: HPC * HD * (c + 1), :].T),
                "cosz": CH_c,
                "sinz": SH_c,
                "swp": S_c,
                "iden": id_c,
                "mask": mask,
            }
        )
    res = run_bass_kernel_spmd(nc, in_maps, core_ids=list(range(NCORE)))
    out = np.concatenate([res.results[c]["out"] for c in range(NCORE)], axis=1)
    return np.ascontiguousarray(out, dtype=np.float32)


# revision 19
# speedup vs baseline: 1.2283x; 1.2283x over previous
"""Distributed GQA attention layer (seq=2048, dim=4096, 32 q heads / 8 kv heads,
rope theta=5e5, causal) on 8 TRN2 NeuronCores.

Sharding: tensor-parallel over heads. Core c owns q heads 4c..4c+3 and kv head c.
Each core computes its 4 heads' attention output in transposed layout
attnT_local [512, 2048], an AllGather over the partition axis assembles
attnT_full [4096, 2048], and each core then computes 512 output columns of the
final projection: out_c [2048, 512] = attnT_full.T @ woT_c.  The host
concatenates the 8 column blocks.

Device-side layout tricks (all host-prepped):
 - x fed transposed [dim, seq]; all weights fed as [dim(contract), out].
 - rope pairs (2i, 2i+1) are permuted to half-split form by permuting wq/wk
   rows, making rope a "rotate-half": r = t*CH + swap_halves(t)*SH, where
   swap_halves is a 128x128 permutation matmul and CH/SH are [128, seq]
   tables. The q·k inner product is invariant under the shared permutation.
 - 1/sqrt(hd) folded into wq.
 - softmax runs unnormalized; a ones-column appended to v makes the PV matmul
   accumulate the denominator in psum column 128, and the normalization is
   fused into the psum->sbuf copy as a per-partition activation scale.
"""
import sys

sys.path.insert(0, "/opt/trn_rl_repo")
import numpy as np

import concourse.bass as bass
import concourse.mybir as mybir
import concourse.tile as tile
from concourse import bacc
from concourse.bass_utils import run_bass_kernel_spmd

import os as _os
SEQ = int(_os.environ.get("KERN_SEQ", "2048"))
DIM = int(_os.environ.get("KERN_DIM", "4096"))
NH, NKV, HD = 32, 8, 128
THETA = 500000.0
NCORE = 8
HPC = NH // NCORE          # 4 q heads per core
HALF = HD // 2
SCALE = 1.0 / np.sqrt(HD)
NDT = DIM // 128           # contraction tiles for qkv projections
NET = (NH * HD) // 128     # e-dim tiles for the output projection (32)
NCH = SEQ // 512           # 4 seq chunks
F32 = mybir.dt.float32
AF = mybir.ActivationFunctionType
ALU = mybir.AluOpType

import os
COMPUTE = os.environ.get("KERN_COMPUTE", "bf16")  # f32 | f32r | bf16
if COMPUTE == "bf16":
    CD = mybir.dt.bfloat16
else:
    CD = F32


def _mm(ap):
    """Matmul-operand view: reinterpret f32 tiles as float32r when requested."""
    if COMPUTE == "f32r":
        return ap.bitcast(mybir.dt.float32r)
    return ap


def build():
    nc = bacc.Bacc("TRN2", target_bir_lowering=False, debug=False, num_devices=NCORE)
    x_e = nc.dram_tensor("x", [DIM, SEQ], CD, kind="ExternalInput")
    wq_e = nc.dram_tensor("wq", [DIM, HPC * HD], CD, kind="ExternalInput")
    wk_e = nc.dram_tensor("wk", [DIM, HD], CD, kind="ExternalInput")
    wv_e = nc.dram_tensor("wv", [DIM, HD], CD, kind="ExternalInput")
    wo_e = nc.dram_tensor("wo", [NH * HD, HPC * HD], CD, kind="ExternalInput")
    ch_e = nc.dram_tensor("cosz", [HD, SEQ], CD, kind="ExternalInput")
    sh_e = nc.dram_tensor("sinz", [HD, SEQ], CD, kind="ExternalInput")
    sw_e = nc.dram_tensor("swp", [HD, HD], CD, kind="ExternalInput")
    id_e = nc.dram_tensor("iden", [HD, HD], CD, kind="ExternalInput")
    mk_e = nc.dram_tensor("mask", [HD, HD], F32, kind="ExternalInput")
    out_e = nc.dram_tensor("out", [SEQ, HPC * HD], F32, kind="ExternalOutput")

    agin = [nc.dram_tensor(f"agin{j}", [HPC * HD, 512], CD) for j in range(NCH)]
    agout = [
        nc.dram_tensor(f"agout{j}", [NH * HD, 512], CD, addr_space="Shared")
        for j in range(NCH)
    ]

    with tile.TileContext(nc) as tc:
        _build_body(nc, tc, locals())
    nc.compile()
    return nc


def _build_body(nc, tc, ext):
    from contextlib import ExitStack

    x_e, wq_e, wk_e, wv_e, wo_e = (ext[k] for k in ("x_e", "wq_e", "wk_e", "wv_e", "wo_e"))
    ch_e, sh_e, sw_e, id_e, mk_e = (ext[k] for k in ("ch_e", "sh_e", "sw_e", "id_e", "mk_e"))
    out_e, agin, agout = ext["out_e"], ext["agin"], ext["agout"]

    with ExitStack() as ctx:
        consts = ctx.enter_context(tc.tile_pool(name="consts", bufs=1))
        qkv = ctx.enter_context(tc.tile_pool(name="qkv", bufs=1))
        rope = ctx.enter_context(tc.tile_pool(name="rope", bufs=2))
        epool = ctx.enter_context(tc.tile_pool(name="epool", bufs=3))
        atp = ctx.enter_context(tc.tile_pool(name="atp", bufs=2))
        small = ctx.enter_context(tc.tile_pool(name="small", bufs=2))
        ocp = ctx.enter_context(tc.tile_pool(name="ocp", bufs=2))
        ps512 = ctx.enter_context(tc.tile_pool(name="ps512", bufs=3, space="PSUM"))
        psbank = ctx.enter_context(tc.tile_pool(name="psbank", bufs=4, space="PSUM"))
        pstr = ctx.enter_context(tc.tile_pool(name="pstr", bufs=1, space="PSUM"))

        # ---- constants / persistent tensors ----
        ch_sb = consts.tile([HD, SEQ], CD, name="ch_sb")
        sh_sb = consts.tile([HD, SEQ], CD, name="sh_sb")
        sw_sb = consts.tile([HD, HD], CD, name="sw_sb")
        nc.sync.dma_start(sw_sb[:], sw_e[:, :])
        id_sb = consts.tile([HD, HD], CD, name="id_sb")
        nc.sync.dma_start(id_sb[:], id_e[:, :])
        mk_sb = consts.tile([HD, HD], F32, name="mk_sb")
        nc.sync.dma_start(mk_sb[:], mk_e[:, :])
        # resident weights: wq [d-tile, m] blocks, wk/wv per d-tile, wo per e-tile
        # (loads are emitted inside P1, interleaved with chunk-0 x tiles, so the
        # first projection matmul starts after a handful of DMAs)
        wq_sb = consts.tile([128, NDT * HPC * 128], CD, name="wq_sb")
        wk_sb = consts.tile([128, NDT * 128], CD, name="wk_sb")
        wv_sb = consts.tile([128, NDT * 128], CD, name="wv_sb")
        wo_sb = consts.tile([128, NET * 512], CD, name="wo_sb")
        qt_sb = qkv.tile([128, HPC * SEQ], CD, name="qt_sb")      # [hd, (h, seq)]
        kt_sb = qkv.tile([128, SEQ], CD, name="kt_sb")            # [hd, seq]
        v_sb = qkv.tile([128, (SEQ // 128) * (HD + 1)], CD, name="v_sb")  # [seqP,(t,129)]
        # ones column of v-hat (col 128 of each 129-block)
        nc.vector.memset(
            v_sb[:].rearrange("p (t c) -> p t c", c=HD + 1)[:, :, HD:HD + 1], 1.0
        )

        # ================= P1: projections + rope =================
        with tc.tile_pool(name="xin", bufs=34) as xin:
            for j in range(NCH):
                xts = []
                for d in range(NDT):
                    xt = xin.tile([128, 512], CD, tag="xin", name=f"x_{j}_{d}")
                    nc.sync.dma_start(xt[:], x_e[128 * d:128 * (d + 1), 512 * j:512 * (j + 1)])
                    xts.append(xt)
                    if j == 0:
                        nc.sync.dma_start(
                            wq_sb[:, 512 * d:512 * (d + 1)], wq_e[128 * d:128 * (d + 1), :]
                        )
                        nc.sync.dma_start(
                            wk_sb[:, 128 * d:128 * (d + 1)], wk_e[128 * d:128 * (d + 1), :]
                        )
                        nc.sync.dma_start(
                            wv_sb[:, 128 * d:128 * (d + 1)], wv_e[128 * d:128 * (d + 1), :]
                        )
                if j == 0:
                    nc.sync.dma_start(ch_sb[:], ch_e[:, :])
                    nc.sync.dma_start(sh_sb[:], sh_e[:, :])
                if j == min(1, NCH - 1):
                    for et in range(NET):
                        nc.sync.dma_start(
                            wo_sb[:, 512 * et:512 * (et + 1)],
                            wo_e[128 * et:128 * (et + 1), :],
                        )
                # q heads (m=0..3) and k (m=4): outputs in [hd, seq] layout
                for m in range(HPC + 1):
                    acc = ps512.tile([128, 512], F32, tag="b512", name=f"acc_{j}_{m}")
                    for d in range(NDT):
                        wsl = (
                            wq_sb[:, 512 * d + 128 * m: 512 * d + 128 * (m + 1)]
                            if m < HPC
                            else wk_sb[:, 128 * d:128 * (d + 1)]
                        )
                        nc.tensor.matmul(
                            acc[:], _mm(wsl), _mm(xts[d][:]), start=(d == 0), stop=(d == NDT - 1)
                        )
                    # rope: dest = acc*CH + (swap @ acc)*SH
                    t_sb = rope.tile([128, 512], CD, tag="tsb", name=f"t_{j}_{m}")
                    nc.scalar.activation(t_sb[:], acc[:], AF.Copy)
                    ups = ps512.tile([128, 512], F32, tag="b512", name=f"u_{j}_{m}")
                    nc.tensor.matmul(ups[:], _mm(sw_sb[:]), _mm(t_sb[:]), start=True, stop=True)
                    m1 = rope.tile([128, 512], CD, tag="m1", name=f"m1_{j}_{m}")
                    nc.vector.tensor_tensor(
                        m1[:], t_sb[:], ch_sb[:, 512 * j:512 * (j + 1)], op=ALU.mult
                    )
                    if m < HPC:
                        dest = qt_sb[:, SEQ * m + 512 * j: SEQ * m + 512 * (j + 1)]
                    else:
                        dest = kt_sb[:, 512 * j:512 * (j + 1)]
                    nc.vector.tensor_tensor(
                        dest, ups[:], sh_sb[:, 512 * j:512 * (j + 1)], op=ALU.mult
                    )
                    nc.vector.tensor_add(dest, dest, m1[:])
                # v: wide-N [hd, seq] projection, then PE-transpose to natural layout
                vt_ps = ps512.tile([128, 512], F32, tag="b512", name=f"vt_{j}")
                for d in range(NDT):
                    nc.tensor.matmul(
                        vt_ps[:],
                        _mm(wv_sb[:, 128 * d:128 * (d + 1)]),
                        _mm(xts[d][:]),
                        start=(d == 0),
                        stop=(d == NDT - 1),
                    )
                vt_sb = rope.tile([128, 512], CD, tag="vtsb", name=f"vt_sb_{j}")
                nc.scalar.activation(vt_sb[:], vt_ps[:], AF.Copy)
                for st in range(4):
                    t = 4 * j + st
                    trv = pstr.tile([128, 128], CD, tag="tr", name=f"trv_{j}_{st}")
                    nc.tensor.transpose(trv[:], vt_sb[:, 128 * st:128 * (st + 1)], id_sb[:])
                    nc.vector.tensor_copy(
                        v_sb[:, (HD + 1) * t:(HD + 1) * t + HD], trv[:]
                    )

        # ================= P2/P3: attention + AG + out-proj =================
        with tc.tile_pool(name="wop", bufs=1) as wop, tc.tile_pool(name="agp", bufs=12) as agp:
            def emit_wo(j):
                wops = [
                    psbank.tile([128, 512], F32, tag="bank", name=f"wop_{j}_{sq}")
                    for sq in range(4)
                ]
                for et in range(NET):
                    agt = agp.tile([128, 512], CD, tag="agt", name=f"ag_{j}_{et}")
                    nc.sync.dma_start(agt[:], agout[j][128 * et:128 * (et + 1), :])
                    for sq in range(4):
                        nc.tensor.matmul(
                            wops[sq][:],
                            _mm(agt[:, 128 * sq:128 * (sq + 1)]),
                            _mm(wo_sb[:, 512 * et:512 * (et + 1)]),
                            start=(et == 0),
                            stop=(et == NET - 1),
                        )
                for sq in range(4):
                    oc = ocp.tile([128, 512], F32, tag="oc", name=f"oc_{j}_{sq}")
                    nc.scalar.activation(oc[:], wops[sq][:], AF.Copy)
                    nc.sync.dma_start(
                        out_e[512 * j + 128 * sq: 512 * j + 128 * (sq + 1), :], oc[:]
                    )

            for j in range(NCH):
                # ---- attention for seq chunk j, all 4 heads ----
                at_sb = atp.tile([128, HPC * 512], CD, tag="atT", name=f"atT{j}")
                for h in range(HPC):
                    qsl = qt_sb[:, SEQ * h + 512 * j: SEQ * h + 512 * (j + 1)]
                    aps = [
                        psbank.tile([128, HD + 1], F32, tag="bank", name=f"ap_{j}_{h}_{sq}")
                        for sq in range(4)
                    ]
                    for skt in range(4 * j + 4):
                        r = skt - 4 * j
                        lo = 128 * r if r > 0 else 0
                        stp = ps512.tile([128, 512], F32, tag="b512", name=f"st_{j}_{h}_{skt}")
                        nc.tensor.matmul(
                            stp[:, lo:512],
                            _mm(kt_sb[:, 128 * skt:128 * (skt + 1)]),
                            _mm(qsl[:, lo:512]),
                            start=True, stop=True,
                        )
                        E = epool.tile([128, 512], CD, tag="E", name=f"E_{j}_{h}_{skt}")
                        if r >= 0:
                            mt = small.tile([128, 128], F32, tag="mt", name=f"mt_{j}_{h}_{skt}")
                            nc.vector.tensor_add(mt[:], stp[:, 128 * r:128 * (r + 1)], mk_sb[:])
                            nc.scalar.activation(E[:, 128 * r:128 * (r + 1)], mt[:], AF.Exp)
                            if r < 3:
                                nc.scalar.activation(
                                    E[:, 128 * (r + 1):512], stp[:, 128 * (r + 1):512], AF.Exp
                                )
                        else:
                            nc.scalar.activation(E[:], stp[:], AF.Exp)
                        for sq in range(max(0, r), 4):
                            nc.tensor.matmul(
                                aps[sq][:],
                                _mm(E[:, 128 * sq:128 * (sq + 1)]),
                                _mm(v_sb[:, (HD + 1) * skt:(HD + 1) * (skt + 1)]),
                                start=(skt == 0),
                                stop=(skt == 4 * j + sq),
                            )
                            if skt == 4 * j + sq:  # this sq-subtile is complete
                                inv = small.tile([128, 1], F32, tag="inv", name=f"i_{j}_{h}_{sq}")
                                nc.vector.reciprocal(inv[:], aps[sq][:, HD:HD + 1])
                                an = small.tile([128, 128], CD, tag="an", name=f"an_{j}_{h}_{sq}")
                                nc.vector.tensor_scalar_mul(
                                    an[:], aps[sq][:, 0:HD], inv[:]
                                )
                                trp = pstr.tile([128, 128], CD, tag="tr", name=f"tr_{j}_{h}_{sq}")
                                nc.tensor.transpose(trp[:], an[:], id_sb[:])
                                nc.vector.tensor_copy(
                                    at_sb[:, 512 * h + 128 * sq: 512 * h + 128 * (sq + 1)],
                                    trp[:],
                                )
                # ---- AllGather chunk j ----
                nc.sync.dma_start(
                    agin[j][:, :].rearrange("(h p) s -> p h s", h=HPC),
                    at_sb[:].rearrange("p (h s) -> p h s", h=HPC),
                )
                nc.gpsimd.collective_compute(
                    "AllGather",
                    ALU.bypass,
                    replica_groups=[list(range(NCORE))],
                    ins=[agin[j][:, :]],
                    outs=[agout[j][:, :]],
                )
                # ---- out projection, 2 chunks behind (software pipelined so
                # the PE stream never waits on an in-flight AllGather) ----
                if j > 1:
                    emit_wo(j - 2)
            for jj in range(max(0, NCH - 2), NCH):
                emit_wo(jj)


# ---------------- host side ----------------
_PERM = np.concatenate([np.arange(0, HD, 2), np.arange(1, HD, 2)])
_NC_CACHE = {}


def _get_nc():
    if "nc" not in _NC_CACHE:
        _NC_CACHE["nc"] = build()
    return _NC_CACHE["nc"]


def _prep_consts():
    freqs = 1.0 / (THETA ** (np.arange(HALF, dtype=np.float64) / HALF))
    ang = np.arange(SEQ, dtype=np.float64)[:, None] * freqs[None, :]
    cos = np.cos(ang).astype(np.float32)
    sin = np.sin(ang).astype(np.float32)
    CH = np.ascontiguousarray(np.concatenate([cos, cos], axis=1).T)
    SH = np.ascontiguousarray(np.concatenate([-sin, sin], axis=1).T)
    S_l = np.zeros((HD, HD), np.float32)
    for i in range(HD):
        S_l[(i + 64) % HD, i] = 1.0
    iden = np.eye(HD, dtype=np.float32)
    mask = np.where(
        np.arange(HD)[:, None] <= np.arange(HD)[None, :], 0.0, -1e30
    ).astype(np.float32)
    return CH, SH, S_l, iden, mask


def _cd(a):
    if COMPUTE == "bf16":
        import ml_dtypes
        return np.ascontiguousarray(a).astype(ml_dtypes.bfloat16)
    return np.ascontiguousarray(a, dtype=np.float32)


def kernel(x, wq, wk, wv, wo):
    x, wq, wk, wv, wo = (np.asarray(a, dtype=np.float32) for a in (x, wq, wk, wv, wo))
    nc = _get_nc()
    CH, SH, S_l, iden, mask = _prep_consts()
    xT = np.ascontiguousarray(x.T)
    wq_p = wq.reshape(NH, HD, DIM)[:, _PERM, :] * SCALE
    wk_p = wk.reshape(NKV, HD, DIM)[:, _PERM, :]
    xT_c = _cd(xT)
    CH_c, SH_c, S_c, id_c = _cd(CH), _cd(SH), _cd(S_l), _cd(iden)
    in_maps = []
    for c in range(NCORE):
        in_maps.append(
            {
                "x": xT_c,
                "wq": _cd(wq_p[HPC * c: HPC * (c + 1)].reshape(HPC * HD, DIM).T),
                "wk": _cd(wk_p[c].T),
                "wv": _cd(wv[HD * c: HD * (c + 1), :].T),
                "wo": _cd(wo[HPC * HD * c: HPC * HD * (c + 1), :].T),
                "cosz": CH_c,
                "sinz": SH_c,
                "swp": S_c,
                "iden": id_c,
                "mask": mask,
            }
        )
    res = run_bass_kernel_spmd(nc, in_maps, core_ids=list(range(NCORE)))
    out = np.concatenate([res.results[c]["out"] for c in range(NCORE)], axis=1)
    return np.ascontiguousarray(out, dtype=np.float32)


# revision 20
# speedup vs baseline: 1.3151x; 1.0706x over previous
"""Distributed GQA attention layer (seq=2048, dim=4096, 32 q heads / 8 kv heads,
rope theta=5e5, causal) on 8 TRN2 NeuronCores.

Sharding: tensor-parallel over heads. Core c owns q heads 4c..4c+3 and kv head c.
Each core computes its 4 heads' attention output in transposed layout
attnT_local [512, 2048], an AllGather over the partition axis assembles
attnT_full [4096, 2048], and each core then computes 512 output columns of the
final projection: out_c [2048, 512] = attnT_full.T @ woT_c.  The host
concatenates the 8 column blocks.

Device-side layout tricks (all host-prepped):
 - x fed transposed [dim, seq]; all weights fed as [dim(contract), out].
 - rope pairs (2i, 2i+1) are permuted to half-split form by permuting wq/wk
   rows, making rope a "rotate-half": r = t*CH + swap_halves(t)*SH, where
   swap_halves is a 128x128 permutation matmul and CH/SH are [128, seq]
   tables. The q·k inner product is invariant under the shared permutation.
 - 1/sqrt(hd) folded into wq.
 - softmax runs unnormalized; a ones-column appended to v makes the PV matmul
   accumulate the denominator in psum column 128, and the normalization is
   fused into the psum->sbuf copy as a per-partition activation scale.
"""
import sys

sys.path.insert(0, "/opt/trn_rl_repo")
import numpy as np

import concourse.bass as bass
import concourse.mybir as mybir
import concourse.tile as tile
from concourse import bacc
from concourse.bass_utils import run_bass_kernel_spmd

import os as _os
SEQ = int(_os.environ.get("KERN_SEQ", "2048"))
DIM = int(_os.environ.get("KERN_DIM", "4096"))
NH, NKV, HD = 32, 8, 128
THETA = 500000.0
NCORE = 8
HPC = NH // NCORE          # 4 q heads per core
HALF = HD // 2
SCALE = 1.0 / np.sqrt(HD)
NDT = DIM // 128           # contraction tiles for qkv projections
NET = (NH * HD) // 128     # e-dim tiles for the output projection (32)
NCH = SEQ // 512           # 4 seq chunks
F32 = mybir.dt.float32
AF = mybir.ActivationFunctionType
ALU = mybir.AluOpType

import os
COMPUTE = os.environ.get("KERN_COMPUTE", "bf16")  # f32 | f32r | bf16
if COMPUTE == "bf16":
    CD = mybir.dt.bfloat16
else:
    CD = F32


def _mm(ap):
    """Matmul-operand view: reinterpret f32 tiles as float32r when requested."""
    if COMPUTE == "f32r":
        return ap.bitcast(mybir.dt.float32r)
    return ap


def build():
    nc = bacc.Bacc("TRN2", target_bir_lowering=False, debug=False, num_devices=NCORE)
    x_e = nc.dram_tensor("x", [DIM, SEQ], CD, kind="ExternalInput")
    wq_e = nc.dram_tensor("wq", [DIM, HPC * HD], CD, kind="ExternalInput")
    wk_e = nc.dram_tensor("wk", [DIM, HD], CD, kind="ExternalInput")
    wv_e = nc.dram_tensor("wv", [DIM, HD], CD, kind="ExternalInput")
    wo_e = nc.dram_tensor("wo", [NH * HD, HPC * HD], CD, kind="ExternalInput")
    ch_e = nc.dram_tensor("cosz", [HD, SEQ], CD, kind="ExternalInput")
    sh_e = nc.dram_tensor("sinz", [HD, SEQ], CD, kind="ExternalInput")
    sw_e = nc.dram_tensor("swp", [HD, HD], CD, kind="ExternalInput")
    id_e = nc.dram_tensor("iden", [HD, HD], CD, kind="ExternalInput")
    mk_e = nc.dram_tensor("mask", [HD, HD], F32, kind="ExternalInput")
    out_e = nc.dram_tensor("out", [SEQ, HPC * HD], F32, kind="ExternalOutput")

    agin = [nc.dram_tensor(f"agin{j}", [HPC * HD, 512], CD) for j in range(NCH)]
    agout = [
        nc.dram_tensor(f"agout{j}", [NH * HD, 512], CD, addr_space="Shared")
        for j in range(NCH)
    ]

    with tile.TileContext(nc) as tc:
        _build_body(nc, tc, locals())
    nc.compile()
    return nc


def _build_body(nc, tc, ext):
    from contextlib import ExitStack

    x_e, wq_e, wk_e, wv_e, wo_e = (ext[k] for k in ("x_e", "wq_e", "wk_e", "wv_e", "wo_e"))
    ch_e, sh_e, sw_e, id_e, mk_e = (ext[k] for k in ("ch_e", "sh_e", "sw_e", "id_e", "mk_e"))
    out_e, agin, agout = ext["out_e"], ext["agin"], ext["agout"]

    with ExitStack() as ctx:
        consts = ctx.enter_context(tc.tile_pool(name="consts", bufs=1))
        qkv = ctx.enter_context(tc.tile_pool(name="qkv", bufs=1))
        rope = ctx.enter_context(tc.tile_pool(name="rope", bufs=2))
        epool = ctx.enter_context(tc.tile_pool(name="epool", bufs=3))
        atp = ctx.enter_context(tc.tile_pool(name="atp", bufs=2))
        small = ctx.enter_context(tc.tile_pool(name="small", bufs=2))
        ocp = ctx.enter_context(tc.tile_pool(name="ocp", bufs=2))
        ps512 = ctx.enter_context(tc.tile_pool(name="ps512", bufs=3, space="PSUM"))
        psbank = ctx.enter_context(tc.tile_pool(name="psbank", bufs=4, space="PSUM"))
        pstr = ctx.enter_context(tc.tile_pool(name="pstr", bufs=1, space="PSUM"))

        # ---- constants / persistent tensors ----
        ch_sb = consts.tile([HD, SEQ], CD, name="ch_sb")
        sh_sb = consts.tile([HD, SEQ], CD, name="sh_sb")
        sw_sb = consts.tile([HD, HD], CD, name="sw_sb")
        nc.sync.dma_start(sw_sb[:], sw_e[:, :])
        id_sb = consts.tile([HD, HD], CD, name="id_sb")
        nc.sync.dma_start(id_sb[:], id_e[:, :])
        mk_sb = consts.tile([HD, HD], F32, name="mk_sb")
        nc.sync.dma_start(mk_sb[:], mk_e[:, :])
        # resident weights: wq [d-tile, m] blocks, wk/wv per d-tile, wo per e-tile
        # (loads are emitted inside P1, interleaved with chunk-0 x tiles, so the
        # first projection matmul starts after a handful of DMAs)
        wq_sb = consts.tile([128, NDT * HPC * 128], CD, name="wq_sb")
        wk_sb = consts.tile([128, NDT * 128], CD, name="wk_sb")
        wv_sb = consts.tile([128, NDT * 128], CD, name="wv_sb")
        wo_sb = consts.tile([128, NET * 512], CD, name="wo_sb")
        qt_sb = qkv.tile([128, HPC * SEQ], CD, name="qt_sb")      # [hd, (h, seq)]
        kt_sb = qkv.tile([128, SEQ], CD, name="kt_sb")            # [hd, seq]
        v_sb = qkv.tile([128, (SEQ // 128) * (HD + 1)], CD, name="v_sb")  # [seqP,(t,129)]
        # ones column of v-hat (col 128 of each 129-block)
        nc.vector.memset(
            v_sb[:].rearrange("p (t c) -> p t c", c=HD + 1)[:, :, HD:HD + 1], 1.0
        )

        # ================= P1: projections + rope =================
        with tc.tile_pool(name="xin", bufs=34) as xin:
            for j in range(NCH):
                xts = []
                for d in range(NDT):
                    xt = xin.tile([128, 512], CD, tag="xin", name=f"x_{j}_{d}")
                    nc.sync.dma_start(xt[:], x_e[128 * d:128 * (d + 1), 512 * j:512 * (j + 1)])
                    xts.append(xt)
                    if j == 0:
                        nc.sync.dma_start(
                            wq_sb[:, 512 * d:512 * (d + 1)], wq_e[128 * d:128 * (d + 1), :]
                        )
                        nc.sync.dma_start(
                            wk_sb[:, 128 * d:128 * (d + 1)], wk_e[128 * d:128 * (d + 1), :]
                        )
                        nc.sync.dma_start(
                            wv_sb[:, 128 * d:128 * (d + 1)], wv_e[128 * d:128 * (d + 1), :]
                        )
                if j == 0:
                    nc.sync.dma_start(ch_sb[:], ch_e[:, :])
                    nc.sync.dma_start(sh_sb[:], sh_e[:, :])
                if j == min(1, NCH - 1):
                    for et in range(NET):
                        nc.sync.dma_start(
                            wo_sb[:, 512 * et:512 * (et + 1)],
                            wo_e[128 * et:128 * (et + 1), :],
                        )
                # q heads (m=0..3) and k (m=4): outputs in [hd, seq] layout
                for m in range(HPC + 1):
                    acc = ps512.tile([128, 512], F32, tag="b512", name=f"acc_{j}_{m}")
                    for d in range(NDT):
                        wsl = (
                            wq_sb[:, 512 * d + 128 * m: 512 * d + 128 * (m + 1)]
                            if m < HPC
                            else wk_sb[:, 128 * d:128 * (d + 1)]
                        )
                        nc.tensor.matmul(
                            acc[:], _mm(wsl), _mm(xts[d][:]), start=(d == 0), stop=(d == NDT - 1)
                        )
                    # rope: dest = acc*CH + (swap @ acc)*SH
                    t_sb = rope.tile([128, 512], CD, tag="tsb", name=f"t_{j}_{m}")
                    nc.scalar.activation(t_sb[:], acc[:], AF.Copy)
                    ups = ps512.tile([128, 512], F32, tag="b512", name=f"u_{j}_{m}")
                    nc.tensor.matmul(ups[:], _mm(sw_sb[:]), _mm(t_sb[:]), start=True, stop=True)
                    m1 = rope.tile([128, 512], CD, tag="m1", name=f"m1_{j}_{m}")
                    nc.vector.tensor_tensor(
                        m1[:], t_sb[:], ch_sb[:, 512 * j:512 * (j + 1)], op=ALU.mult
                    )
                    if m < HPC:
                        dest = qt_sb[:, SEQ * m + 512 * j: SEQ * m + 512 * (j + 1)]
                    else:
                        dest = kt_sb[:, 512 * j:512 * (j + 1)]
                    nc.vector.tensor_tensor(
                        dest, ups[:], sh_sb[:, 512 * j:512 * (j + 1)], op=ALU.mult
                    )
                    nc.vector.tensor_add(dest, dest, m1[:])
                # v in natural [seq, hd] layout
                vaccs = [
                    psbank.tile([128, 128], F32, tag="bank", name=f"vacc_{j}_{st}")
                    for st in range(4)
                ]
                for d in range(NDT):
                    for st in range(4):
                        nc.tensor.matmul(
                            vaccs[st][:],
                            _mm(xts[d][:, 128 * st:128 * (st + 1)]),
                            _mm(wv_sb[:, 128 * d:128 * (d + 1)]),
                            start=(d == 0),
                            stop=(d == NDT - 1),
                        )
                for st in range(4):
                    t = 4 * j + st
                    nc.scalar.activation(
                        v_sb[:, (HD + 1) * t:(HD + 1) * t + HD],
                        vaccs[st][:],
                        AF.Copy,
                    )

        # ================= P2/P3: attention + AG + out-proj =================
        with tc.tile_pool(name="wop", bufs=1) as wop, tc.tile_pool(name="agp", bufs=12) as agp:
            def emit_wo(j):
                wops = [
                    psbank.tile([128, 512], F32, tag="bank", name=f"wop_{j}_{sq}")
                    for sq in range(4)
                ]
                for et in range(NET):
                    agt = agp.tile([128, 512], CD, tag="agt", name=f"ag_{j}_{et}")
                    nc.sync.dma_start(agt[:], agout[j][128 * et:128 * (et + 1), :])
                    for sq in range(4):
                        nc.tensor.matmul(
                            wops[sq][:],
                            _mm(agt[:, 128 * sq:128 * (sq + 1)]),
                            _mm(wo_sb[:, 512 * et:512 * (et + 1)]),
                            start=(et == 0),
                            stop=(et == NET - 1),
                        )
                for sq in range(4):
                    oc = ocp.tile([128, 512], F32, tag="oc", name=f"oc_{j}_{sq}")
                    nc.scalar.activation(oc[:], wops[sq][:], AF.Copy)
                    nc.sync.dma_start(
                        out_e[512 * j + 128 * sq: 512 * j + 128 * (sq + 1), :], oc[:]
                    )

            for j in range(NCH):
                # ---- attention for seq chunk j, all 4 heads ----
                at_sb = atp.tile([128, HPC * 512], CD, tag="atT", name=f"atT{j}")
                for h in range(HPC):
                    qsl = qt_sb[:, SEQ * h + 512 * j: SEQ * h + 512 * (j + 1)]
                    aps = [
                        psbank.tile([128, HD + 1], F32, tag="bank", name=f"ap_{j}_{h}_{sq}")
                        for sq in range(4)
                    ]
                    for skt in range(4 * j + 4):
                        r = skt - 4 * j
                        lo = 128 * r if r > 0 else 0
                        stp = ps512.tile([128, 512], F32, tag="b512", name=f"st_{j}_{h}_{skt}")
                        nc.tensor.matmul(
                            stp[:, lo:512],
                            _mm(kt_sb[:, 128 * skt:128 * (skt + 1)]),
                            _mm(qsl[:, lo:512]),
                            start=True, stop=True,
                        )
                        E = epool.tile([128, 512], CD, tag="E", name=f"E_{j}_{h}_{skt}")
                        if r >= 0:
                            mt = small.tile([128, 128], F32, tag="mt", name=f"mt_{j}_{h}_{skt}")
                            nc.vector.tensor_add(mt[:], stp[:, 128 * r:128 * (r + 1)], mk_sb[:])
                            nc.scalar.activation(E[:, 128 * r:128 * (r + 1)], mt[:], AF.Exp)
                            if r < 3:
                                nc.scalar.activation(
                                    E[:, 128 * (r + 1):512], stp[:, 128 * (r + 1):512], AF.Exp
                                )
                        else:
                            nc.scalar.activation(E[:], stp[:], AF.Exp)
                        for sq in range(max(0, r), 4):
                            nc.tensor.matmul(
                                aps[sq][:],
                                _mm(E[:, 128 * sq:128 * (sq + 1)]),
                                _mm(v_sb[:, (HD + 1) * skt:(HD + 1) * (skt + 1)]),
                                start=(skt == 0),
                                stop=(skt == 4 * j + sq),
                            )
                            if skt == 4 * j + sq:  # this sq-subtile is complete
                                inv = small.tile([128, 1], F32, tag="inv", name=f"i_{j}_{h}_{sq}")
                                nc.vector.reciprocal(inv[:], aps[sq][:, HD:HD + 1])
                                an = small.tile([128, 128], CD, tag="an", name=f"an_{j}_{h}_{sq}")
                                nc.vector.tensor_scalar_mul(
                                    an[:], aps[sq][:, 0:HD], inv[:]
                                )
                                trp = pstr.tile([128, 128], CD, tag="tr", name=f"tr_{j}_{h}_{sq}")
                                nc.tensor.transpose(trp[:], an[:], id_sb[:])
                                nc.vector.tensor_copy(
                                    at_sb[:, 512 * h + 128 * sq: 512 * h + 128 * (sq + 1)],
                                    trp[:],
                                )
                # ---- AllGather chunk j ----
                nc.sync.dma_start(
                    agin[j][:, :].rearrange("(h p) s -> p h s", h=HPC),
                    at_sb[:].rearrange("p (h s) -> p h s", h=HPC),
                )
                nc.gpsimd.collective_compute(
                    "AllGather",
                    ALU.bypass,
                    replica_groups=[list(range(NCORE))],
                    ins=[agin[j][:, :]],
                    outs=[agout[j][:, :]],
                )
                # ---- out projection, 2 chunks behind (software pipelined so
                # the PE stream never waits on an in-flight AllGather) ----
                if j > 1:
                    emit_wo(j - 2)
            for jj in range(max(0, NCH - 2), NCH):
                emit_wo(jj)


# ---------------- host side ----------------
_PERM = np.concatenate([np.arange(0, HD, 2), np.arange(1, HD, 2)])
_NC_CACHE = {}


def _get_nc():
    if "nc" not in _NC_CACHE:
        _NC_CACHE["nc"] = build()
    return _NC_CACHE["nc"]


def _prep_consts():
    freqs = 1.0 / (THETA ** (np.arange(HALF, dtype=np.float64) / HALF))
    ang = np.arange(SEQ, dtype=np.float64)[:, None] * freqs[None, :]
    cos = np.cos(ang).astype(np.float32)
    sin = np.sin(ang).astype(np.float32)
    CH = np.ascontiguousarray(np.concatenate([cos, cos], axis=1).T)
    SH = np.ascontiguousarray(np.concatenate([-sin, sin], axis=1).T)
    S_l = np.zeros((HD, HD), np.float32)
    for i in range(HD):
        S_l[(i + 64) % HD, i] = 1.0
    iden = np.eye(HD, dtype=np.float32)
    mask = np.where(
        np.arange(HD)[:, None] <= np.arange(HD)[None, :], 0.0, -1e30
    ).astype(np.float32)
    return CH, SH, S_l, iden, mask


def _cd(a):
    if COMPUTE == "bf16":
        import ml_dtypes
        return np.ascontiguousarray(a).astype(ml_dtypes.bfloat16)
    return np.ascontiguousarray(a, dtype=np.float32)


def kernel(x, wq, wk, wv, wo):
    x, wq, wk, wv, wo = (np.asarray(a, dtype=np.float32) for a in (x, wq, wk, wv, wo))
    nc = _get_nc()
    CH, SH, S_l, iden, mask = _prep_consts()
    xT = np.ascontiguousarray(x.T)
    wq_p = wq.reshape(NH, HD, DIM)[:, _PERM, :] * SCALE
    wk_p = wk.reshape(NKV, HD, DIM)[:, _PERM, :]
    xT_c = _cd(xT)
    CH_c, SH_c, S_c, id_c = _cd(CH), _cd(SH), _cd(S_l), _cd(iden)
    in_maps = []
    for c in range(NCORE):
        in_maps.append(
            {
                "x": xT_c,
                "wq": _cd(wq_p[HPC * c: HPC * (c + 1)].reshape(HPC * HD, DIM).T),
                "wk": _cd(wk_p[c].T),
                "wv": _cd(wv[HD * c: HD * (c + 1), :].T),
                "wo": _cd(wo[HPC * HD * c: HPC * HD * (c + 1), :].T),
                "cosz": CH_c,
                "sinz": SH_c,
                "swp": S_c,
                "iden": id_c,
                "mask": mask,
            }
        )
    res = run_bass_kernel_spmd(nc, in_maps, core_ids=list(range(NCORE)))
    out = np.concatenate([res.results[c]["out"] for c in range(NCORE)], axis=1)
    return np.ascontiguousarray(out, dtype=np.float32)


# revision 21
# speedup vs baseline: 1.3834x; 1.0520x over previous
"""Distributed GQA attention layer (seq=2048, dim=4096, 32 q heads / 8 kv heads,
rope theta=5e5, causal) on 8 TRN2 NeuronCores.

Sharding: tensor-parallel over heads. Core c owns q heads 4c..4c+3 and kv head c.
Each core computes its 4 heads' attention output in transposed layout
attnT_local [512, 2048], an AllGather over the partition axis assembles
attnT_full [4096, 2048], and each core then computes 512 output columns of the
final projection: out_c [2048, 512] = attnT_full.T @ woT_c.  The host
concatenates the 8 column blocks.

Device-side layout tricks (all host-prepped):
 - x fed transposed [dim, seq]; all weights fed as [dim(contract), out].
 - rope pairs (2i, 2i+1) are permuted to half-split form by permuting wq/wk
   rows, making rope a "rotate-half": r = t*CH + swap_halves(t)*SH, where
   swap_halves is a 128x128 permutation matmul and CH/SH are [128, seq]
   tables. The q·k inner product is invariant under the shared permutation.
 - 1/sqrt(hd) folded into wq.
 - softmax runs unnormalized; a ones-column appended to v makes the PV matmul
   accumulate the denominator in psum column 128, and the normalization is
   fused into the psum->sbuf copy as a per-partition activation scale.
"""
import sys

sys.path.insert(0, "/opt/trn_rl_repo")
import numpy as np

import concourse.bass as bass
import concourse.mybir as mybir
import concourse.tile as tile
from concourse import bacc
from concourse.bass_utils import run_bass_kernel_spmd

import os as _os
SEQ = int(_os.environ.get("KERN_SEQ", "2048"))
DIM = int(_os.environ.get("KERN_DIM", "4096"))
NH, NKV, HD = 32, 8, 128
THETA = 500000.0
NCORE = 8
HPC = NH // NCORE          # 4 q heads per core
HALF = HD // 2
SCALE = 1.0 / np.sqrt(HD)
NDT = DIM // 128           # contraction tiles for qkv projections
NET = (NH * HD) // 128     # e-dim tiles for the output projection (32)
NCH = SEQ // 512           # 4 seq chunks
F32 = mybir.dt.float32
AF = mybir.ActivationFunctionType
ALU = mybir.AluOpType

import os
COMPUTE = os.environ.get("KERN_COMPUTE", "bf16")  # f32 | f32r | bf16
if COMPUTE == "bf16":
    CD = mybir.dt.bfloat16
else:
    CD = F32


def _mm(ap):
    """Matmul-operand view: reinterpret f32 tiles as float32r when requested."""
    if COMPUTE == "f32r":
        return ap.bitcast(mybir.dt.float32r)
    return ap


def build():
    nc = bacc.Bacc("TRN2", target_bir_lowering=False, debug=False, num_devices=NCORE)
    x_e = nc.dram_tensor("x", [DIM, SEQ], CD, kind="ExternalInput")
    wq_e = nc.dram_tensor("wq", [DIM, HPC * HD], CD, kind="ExternalInput")
    wk_e = nc.dram_tensor("wk", [DIM, HD], CD, kind="ExternalInput")
    wv_e = nc.dram_tensor("wv", [DIM, HD], CD, kind="ExternalInput")
    wo_e = nc.dram_tensor("wo", [NH * HD, HPC * HD], CD, kind="ExternalInput")
    ch_e = nc.dram_tensor("cosz", [HD, SEQ], CD, kind="ExternalInput")
    sh_e = nc.dram_tensor("sinz", [HD, SEQ], CD, kind="ExternalInput")
    sw_e = nc.dram_tensor("swp", [HD, HD], CD, kind="ExternalInput")
    id_e = nc.dram_tensor("iden", [HD, HD], CD, kind="ExternalInput")
    mk_e = nc.dram_tensor("mask", [HD, HD], F32, kind="ExternalInput")
    out_e = nc.dram_tensor("out", [SEQ, HPC * HD], F32, kind="ExternalOutput")

    agin = [nc.dram_tensor(f"agin{j}", [HPC * HD, 512], CD) for j in range(NCH)]
    agout = [
        nc.dram_tensor(f"agout{j}", [NH * HD, 512], CD, addr_space="Shared")
        for j in range(NCH)
    ]

    with tile.TileContext(nc) as tc:
        _build_body(nc, tc, locals())
    nc.compile()
    return nc


def _build_body(nc, tc, ext):
    from contextlib import ExitStack

    x_e, wq_e, wk_e, wv_e, wo_e = (ext[k] for k in ("x_e", "wq_e", "wk_e", "wv_e", "wo_e"))
    ch_e, sh_e, sw_e, id_e, mk_e = (ext[k] for k in ("ch_e", "sh_e", "sw_e", "id_e", "mk_e"))
    out_e, agin, agout = ext["out_e"], ext["agin"], ext["agout"]

    with ExitStack() as ctx:
        consts = ctx.enter_context(tc.tile_pool(name="consts", bufs=1))
        qkv = ctx.enter_context(tc.tile_pool(name="qkv", bufs=1))
        rope = ctx.enter_context(tc.tile_pool(name="rope", bufs=2))
        epool = ctx.enter_context(tc.tile_pool(name="epool", bufs=3))
        atp = ctx.enter_context(tc.tile_pool(name="atp", bufs=2))
        small = ctx.enter_context(tc.tile_pool(name="small", bufs=2))
        ocp = ctx.enter_context(tc.tile_pool(name="ocp", bufs=2))
        ps512 = ctx.enter_context(tc.tile_pool(name="ps512", bufs=3, space="PSUM"))
        psbank = ctx.enter_context(tc.tile_pool(name="psbank", bufs=4, space="PSUM"))
        pstr = ctx.enter_context(tc.tile_pool(name="pstr", bufs=1, space="PSUM"))

        # ---- constants / persistent tensors ----
        ch_sb = consts.tile([HD, SEQ], CD, name="ch_sb")
        sh_sb = consts.tile([HD, SEQ], CD, name="sh_sb")
        sw_sb = consts.tile([HD, HD], CD, name="sw_sb")
        nc.sync.dma_start(sw_sb[:], sw_e[:, :])
        id_sb = consts.tile([HD, HD], CD, name="id_sb")
        nc.sync.dma_start(id_sb[:], id_e[:, :])
        mk_sb = consts.tile([HD, HD], F32, name="mk_sb")
        nc.sync.dma_start(mk_sb[:], mk_e[:, :])
        # resident weights: wq [d-tile, m] blocks, wk/wv per d-tile, wo per e-tile
        # (loads are emitted inside P1, interleaved with chunk-0 x tiles, so the
        # first projection matmul starts after a handful of DMAs)
        wq_sb = consts.tile([128, NDT * HPC * 128], CD, name="wq_sb")
        wk_sb = consts.tile([128, NDT * 128], CD, name="wk_sb")
        wv_sb = consts.tile([128, NDT * 128], CD, name="wv_sb")
        wo_sb = consts.tile([128, NET * 512], CD, name="wo_sb")
        qt_sb = qkv.tile([128, HPC * SEQ], CD, name="qt_sb")      # [hd, (h, seq)]
        kt_sb = qkv.tile([128, SEQ], CD, name="kt_sb")            # [hd, seq]
        v_sb = qkv.tile([128, (SEQ // 128) * (HD + 1)], CD, name="v_sb")  # [seqP,(t,129)]
        # ones column of v-hat (col 128 of each 129-block)
        nc.vector.memset(
            v_sb[:].rearrange("p (t c) -> p t c", c=HD + 1)[:, :, HD:HD + 1], 1.0
        )

        # ================= P1: projections + rope =================
        with tc.tile_pool(name="xin", bufs=34) as xin:
            for j in range(NCH):
                xts = []
                for d in range(NDT):
                    xt = xin.tile([128, 512], CD, tag="xin", name=f"x_{j}_{d}")
                    nc.sync.dma_start(xt[:], x_e[128 * d:128 * (d + 1), 512 * j:512 * (j + 1)])
                    xts.append(xt)
                    if j == 0:
                        nc.sync.dma_start(
                            wq_sb[:, 512 * d:512 * (d + 1)], wq_e[128 * d:128 * (d + 1), :]
                        )
                        nc.sync.dma_start(
                            wk_sb[:, 128 * d:128 * (d + 1)], wk_e[128 * d:128 * (d + 1), :]
                        )
                        nc.sync.dma_start(
                            wv_sb[:, 128 * d:128 * (d + 1)], wv_e[128 * d:128 * (d + 1), :]
                        )
                if j == 0:
                    nc.sync.dma_start(ch_sb[:], ch_e[:, :])
                    nc.sync.dma_start(sh_sb[:], sh_e[:, :])
                if j == min(1, NCH - 1):
                    for et in range(NET):
                        nc.sync.dma_start(
                            wo_sb[:, 512 * et:512 * (et + 1)],
                            wo_e[128 * et:128 * (et + 1), :],
                        )
                # q heads (m=0..3) and k (m=4): outputs in [hd, seq] layout
                def emit_rope(m, acc):
                    t_sb = rope.tile([128, 512], CD, tag="tsb", name=f"t_{j}_{m}")
                    nc.scalar.activation(t_sb[:], acc[:], AF.Copy)
                    ups = ps512.tile([128, 512], F32, tag="b512", name=f"u_{j}_{m}")
                    nc.tensor.matmul(ups[:], _mm(sw_sb[:]), _mm(t_sb[:]), start=True, stop=True)
                    m1 = rope.tile([128, 512], CD, tag="m1", name=f"m1_{j}_{m}")
                    nc.vector.tensor_tensor(
                        m1[:], t_sb[:], ch_sb[:, 512 * j:512 * (j + 1)], op=ALU.mult
                    )
                    if m < HPC:
                        dest = qt_sb[:, SEQ * m + 512 * j: SEQ * m + 512 * (j + 1)]
                    else:
                        dest = kt_sb[:, 512 * j:512 * (j + 1)]
                    nc.vector.tensor_tensor(
                        dest, ups[:], sh_sb[:, 512 * j:512 * (j + 1)], op=ALU.mult
                    )
                    nc.vector.tensor_add(dest, dest, m1[:])

                def wslice(m, d):
                    return (
                        wq_sb[:, 512 * d + 128 * m: 512 * d + 128 * (m + 1)]
                        if m < HPC
                        else wk_sb[:, 128 * d:128 * (d + 1)]
                    )

                if j == 0:
                    # d-outer with multi-bank accumulation: matmuls start as soon
                    # as the first x tile lands instead of after all 32 DMAs
                    qaccs = [
                        psbank.tile([128, 512], F32, tag="bank", name=f"dacc_{m}")
                        for m in range(HPC)
                    ]
                    kacc = ps512.tile([128, 512], F32, tag="b512", name="kacc0")
                    for d in range(NDT):
                        for m in range(HPC):
                            nc.tensor.matmul(
                                qaccs[m][:], _mm(wslice(m, d)), _mm(xts[d][:]),
                                start=(d == 0), stop=(d == NDT - 1),
                            )
                        nc.tensor.matmul(
                            kacc[:], _mm(wslice(HPC, d)), _mm(xts[d][:]),
                            start=(d == 0), stop=(d == NDT - 1),
                        )
                    for m in range(HPC):
                        emit_rope(m, qaccs[m])
                    emit_rope(HPC, kacc)
                else:
                    for m in range(HPC + 1):
                        acc = ps512.tile([128, 512], F32, tag="b512", name=f"acc_{j}_{m}")
                        for d in range(NDT):
                            nc.tensor.matmul(
                                acc[:], _mm(wslice(m, d)), _mm(xts[d][:]),
                                start=(d == 0), stop=(d == NDT - 1),
                            )
                        emit_rope(m, acc)
                # v in natural [seq, hd] layout
                vaccs = [
                    psbank.tile([128, 128], F32, tag="bank", name=f"vacc_{j}_{st}")
                    for st in range(4)
                ]
                for d in range(NDT):
                    for st in range(4):
                        nc.tensor.matmul(
                            vaccs[st][:],
                            _mm(xts[d][:, 128 * st:128 * (st + 1)]),
                            _mm(wv_sb[:, 128 * d:128 * (d + 1)]),
                            start=(d == 0),
                            stop=(d == NDT - 1),
                        )
                for st in range(4):
                    t = 4 * j + st
                    nc.scalar.activation(
                        v_sb[:, (HD + 1) * t:(HD + 1) * t + HD],
                        vaccs[st][:],
                        AF.Copy,
                    )

        # ================= P2/P3: attention + AG + out-proj =================
        with tc.tile_pool(name="wop", bufs=1) as wop, tc.tile_pool(name="agp", bufs=12) as agp:
            def emit_wo(j):
                wops = [
                    psbank.tile([128, 512], F32, tag="bank", name=f"wop_{j}_{sq}")
                    for sq in range(4)
                ]
                for et in range(NET):
                    agt = agp.tile([128, 512], CD, tag="agt", name=f"ag_{j}_{et}")
                    nc.sync.dma_start(agt[:], agout[j][128 * et:128 * (et + 1), :])
                    for sq in range(4):
                        nc.tensor.matmul(
                            wops[sq][:],
                            _mm(agt[:, 128 * sq:128 * (sq + 1)]),
                            _mm(wo_sb[:, 512 * et:512 * (et + 1)]),
                            start=(et == 0),
                            stop=(et == NET - 1),
                        )
                for sq in range(4):
                    oc = ocp.tile([128, 512], F32, tag="oc", name=f"oc_{j}_{sq}")
                    nc.scalar.activation(oc[:], wops[sq][:], AF.Copy)
                    nc.sync.dma_start(
                        out_e[512 * j + 128 * sq: 512 * j + 128 * (sq + 1), :], oc[:]
                    )

            for j in range(NCH):
                # ---- attention for seq chunk j, all 4 heads ----
                at_sb = atp.tile([128, HPC * 512], CD, tag="atT", name=f"atT{j}")
                for h in range(HPC):
                    qsl = qt_sb[:, SEQ * h + 512 * j: SEQ * h + 512 * (j + 1)]
                    aps = [
                        psbank.tile([128, HD + 1], F32, tag="bank", name=f"ap_{j}_{h}_{sq}")
                        for sq in range(4)
                    ]
                    for skt in range(4 * j + 4):
                        r = skt - 4 * j
                        lo = 128 * r if r > 0 else 0
                        stp = ps512.tile([128, 512], F32, tag="b512", name=f"st_{j}_{h}_{skt}")
                        nc.tensor.matmul(
                            stp[:, lo:512],
                            _mm(kt_sb[:, 128 * skt:128 * (skt + 1)]),
                            _mm(qsl[:, lo:512]),
                            start=True, stop=True,
                        )
                        E = epool.tile([128, 512], CD, tag="E", name=f"E_{j}_{h}_{skt}")
                        if r >= 0:
                            nc.vector.tensor_add(
                                stp[:, 128 * r:128 * (r + 1)],
                                stp[:, 128 * r:128 * (r + 1)],
                                mk_sb[:],
                            )
                            nc.scalar.activation(E[:, lo:512], stp[:, lo:512], AF.Exp)
                        else:
                            nc.scalar.activation(E[:], stp[:], AF.Exp)
                        for sq in range(max(0, r), 4):
                            nc.tensor.matmul(
                                aps[sq][:],
                                _mm(E[:, 128 * sq:128 * (sq + 1)]),
                                _mm(v_sb[:, (HD + 1) * skt:(HD + 1) * (skt + 1)]),
                                start=(skt == 0),
                                stop=(skt == 4 * j + sq),
                            )
                            if skt == 4 * j + sq:  # this sq-subtile is complete
                                inv = small.tile([128, 1], F32, tag="inv", name=f"i_{j}_{h}_{sq}")
                                nc.vector.reciprocal(inv[:], aps[sq][:, HD:HD + 1])
                                an = small.tile([128, 128], CD, tag="an", name=f"an_{j}_{h}_{sq}")
                                nc.vector.tensor_scalar_mul(
                                    an[:], aps[sq][:, 0:HD], inv[:]
                                )
                                trp = pstr.tile([128, 128], CD, tag="tr", name=f"tr_{j}_{h}_{sq}")
                                nc.tensor.transpose(trp[:], an[:], id_sb[:])
                                nc.vector.tensor_copy(
                                    at_sb[:, 512 * h + 128 * sq: 512 * h + 128 * (sq + 1)],
                                    trp[:],
                                )
                # ---- AllGather chunk j ----
                nc.sync.dma_start(
                    agin[j][:, :].rearrange("(h p) s -> p h s", h=HPC),
                    at_sb[:].rearrange("p (h s) -> p h s", h=HPC),
                )
                nc.gpsimd.collective_compute(
                    "AllGather",
                    ALU.bypass,
                    replica_groups=[list(range(NCORE))],
                    ins=[agin[j][:, :]],
                    outs=[agout[j][:, :]],
                )
                # ---- out projection, 2 chunks behind (software pipelined so
                # the PE stream never waits on an in-flight AllGather) ----
                if j > 1:
                    emit_wo(j - 2)
            for jj in range(max(0, NCH - 2), NCH):
                emit_wo(jj)


# ---------------- host side ----------------
_PERM = np.concatenate([np.arange(0, HD, 2), np.arange(1, HD, 2)])
_NC_CACHE = {}


def _get_nc():
    if "nc" not in _NC_CACHE:
        _NC_CACHE["nc"] = build()
    return _NC_CACHE["nc"]


def _prep_consts():
    freqs = 1.0 / (THETA ** (np.arange(HALF, dtype=np.float64) / HALF))
    ang = np.arange(SEQ, dtype=np.float64)[:, None] * freqs[None, :]
    cos = np.cos(ang).astype(np.float32)
    sin = np.sin(ang).astype(np.float32)
    CH = np.ascontiguousarray(np.concatenate([cos, cos], axis=1).T)
    SH = np.ascontiguousarray(np.concatenate([-sin, sin], axis=1).T)
    S_l = np.zeros((HD, HD), np.float32)
    for i in range(HD):
        S_l[(i + 64) % HD, i] = 1.0
    iden = np.eye(HD, dtype=np.float32)
    mask = np.where(
        np.arange(HD)[:, None] <= np.arange(HD)[None, :], 0.0, -1e30
    ).astype(np.float32)
    return CH, SH, S_l, iden, mask


def _cd(a):
    if COMPUTE == "bf16":
        import ml_dtypes
        return np.ascontiguousarray(a).astype(ml_dtypes.bfloat16)
    return np.ascontiguousarray(a, dtype=np.float32)


def kernel(x, wq, wk, wv, wo):
    x, wq, wk, wv, wo = (np.asarray(a, dtype=np.float32) for a in (x, wq, wk, wv, wo))
    nc = _get_nc()
    CH, SH, S_l, iden, mask = _prep_consts()
    xT = np.ascontiguousarray(x.T)
    wq_p = wq.reshape(NH, HD, DIM)[:, _PERM, :] * SCALE
    wk_p = wk.reshape(NKV, HD, DIM)[:, _PERM, :]
    xT_c = _cd(xT)
    CH_c, SH_c, S_c, id_c = _cd(CH), _cd(SH), _cd(S_l), _cd(iden)
    in_maps = []
    for c in range(NCORE):
        in_maps.append(
            {
                "x": xT_c,
                "wq": _cd(wq_p[HPC * c: HPC * (c + 1)].reshape(HPC * HD, DIM).T),
                "wk": _cd(wk_p[c].T),
                "wv": _cd(wv[HD * c: HD * (c + 1), :].T),
                "wo": _cd(wo[HPC * HD * c: HPC * HD * (c + 1), :].T),
                "cosz": CH_c,
                "sinz": SH_c,
                "swp": S_c,
                "iden": id_c,
                "mask": mask,
            }
        )
    res = run_bass_kernel_spmd(nc, in_maps, core_ids=list(range(NCORE)))
    out = np.concatenate([res.results[c]["out"] for c in range(NCORE)], axis=1)
    return np.ascontiguousarray(out, dtype=np.float32)


# revision 22
# speedup vs baseline: 1.3998x; 1.0118x over previous
"""Distributed GQA attention layer (seq=2048, dim=4096, 32 q heads / 8 kv heads,
rope theta=5e5, causal) on 8 TRN2 NeuronCores.

Sharding: tensor-parallel over heads. Core c owns q heads 4c..4c+3 and kv head c.
Each core computes its 4 heads' attention output in transposed layout
attnT_local [512, 2048], an AllGather over the partition axis assembles
attnT_full [4096, 2048], and each core then computes 512 output columns of the
final projection: out_c [2048, 512] = attnT_full.T @ woT_c.  The host
concatenates the 8 column blocks.

Device-side layout tricks (all host-prepped):
 - x fed transposed [dim, seq]; all weights fed as [dim(contract), out].
 - rope pairs (2i, 2i+1) are permuted to half-split form by permuting wq/wk
   rows, making rope a "rotate-half": r = t*CH + swap_halves(t)*SH, where
   swap_halves is a 128x128 permutation matmul and CH/SH are [128, seq]
   tables. The q·k inner product is invariant under the shared permutation.
 - 1/sqrt(hd) folded into wq.
 - softmax runs unnormalized; a ones-column appended to v makes the PV matmul
   accumulate the denominator in psum column 128, and the normalization is
   fused into the psum->sbuf copy as a per-partition activation scale.
"""
import sys

sys.path.insert(0, "/opt/trn_rl_repo")
import numpy as np

import concourse.bass as bass
import concourse.mybir as mybir
import concourse.tile as tile
from concourse import bacc
from concourse.bass_utils import run_bass_kernel_spmd

import os as _os
SEQ = int(_os.environ.get("KERN_SEQ", "2048"))
DIM = int(_os.environ.get("KERN_DIM", "4096"))
NH, NKV, HD = 32, 8, 128
THETA = 500000.0
NCORE = 8
HPC = NH // NCORE          # 4 q heads per core
HALF = HD // 2
SCALE = 1.0 / np.sqrt(HD)
NDT = DIM // 128           # contraction tiles for qkv projections
NET = (NH * HD) // 128     # e-dim tiles for the output projection (32)
NCH = SEQ // 512           # 4 seq chunks
F32 = mybir.dt.float32
AF = mybir.ActivationFunctionType
ALU = mybir.AluOpType

import os
COMPUTE = os.environ.get("KERN_COMPUTE", "bf16")  # f32 | f32r | bf16
if COMPUTE == "bf16":
    CD = mybir.dt.bfloat16
else:
    CD = F32


def _mm(ap):
    """Matmul-operand view: reinterpret f32 tiles as float32r when requested."""
    if COMPUTE == "f32r":
        return ap.bitcast(mybir.dt.float32r)
    return ap


def build():
    nc = bacc.Bacc("TRN2", target_bir_lowering=False, debug=False, num_devices=NCORE)
    x_e = nc.dram_tensor("x", [DIM, SEQ], CD, kind="ExternalInput")
    wq_e = nc.dram_tensor("wq", [DIM, HPC * HD], CD, kind="ExternalInput")
    wk_e = nc.dram_tensor("wk", [DIM, HD], CD, kind="ExternalInput")
    wv_e = nc.dram_tensor("wv", [DIM, HD], CD, kind="ExternalInput")
    wo_e = nc.dram_tensor("wo", [NH * HD, HPC * HD], CD, kind="ExternalInput")
    ch_e = nc.dram_tensor("cosz", [HD, SEQ], CD, kind="ExternalInput")
    sh_e = nc.dram_tensor("sinz", [HD, SEQ], CD, kind="ExternalInput")
    sw_e = nc.dram_tensor("swp", [HD, HD], CD, kind="ExternalInput")
    id_e = nc.dram_tensor("iden", [HD, HD], CD, kind="ExternalInput")
    mk_e = nc.dram_tensor("mask", [HD, HD], F32, kind="ExternalInput")
    out_e = nc.dram_tensor("out", [SEQ, HPC * HD], F32, kind="ExternalOutput")

    agin = [nc.dram_tensor(f"agin{j}", [HPC * HD, 512], CD) for j in range(NCH)]
    agout = [
        nc.dram_tensor(f"agout{j}", [NH * HD, 512], CD, addr_space="Shared")
        for j in range(NCH)
    ]

    with tile.TileContext(nc) as tc:
        _build_body(nc, tc, locals())
    nc.compile()
    return nc


def _build_body(nc, tc, ext):
    from contextlib import ExitStack

    x_e, wq_e, wk_e, wv_e, wo_e = (ext[k] for k in ("x_e", "wq_e", "wk_e", "wv_e", "wo_e"))
    ch_e, sh_e, sw_e, id_e, mk_e = (ext[k] for k in ("ch_e", "sh_e", "sw_e", "id_e", "mk_e"))
    out_e, agin, agout = ext["out_e"], ext["agin"], ext["agout"]

    with ExitStack() as ctx:
        consts = ctx.enter_context(tc.tile_pool(name="consts", bufs=1))
        qkv = ctx.enter_context(tc.tile_pool(name="qkv", bufs=1))
        rope = ctx.enter_context(tc.tile_pool(name="rope", bufs=2))
        epool = ctx.enter_context(tc.tile_pool(name="epool", bufs=3))
        atp = ctx.enter_context(tc.tile_pool(name="atp", bufs=2))
        small = ctx.enter_context(tc.tile_pool(name="small", bufs=2))
        ocp = ctx.enter_context(tc.tile_pool(name="ocp", bufs=2))
        ps512 = ctx.enter_context(tc.tile_pool(name="ps512", bufs=3, space="PSUM"))
        psbank = ctx.enter_context(tc.tile_pool(name="psbank", bufs=4, space="PSUM"))
        pstr = ctx.enter_context(tc.tile_pool(name="pstr", bufs=1, space="PSUM"))

        # ---- constants / persistent tensors ----
        ch_sb = consts.tile([HD, SEQ], CD, name="ch_sb")
        sh_sb = consts.tile([HD, SEQ], CD, name="sh_sb")
        sw_sb = consts.tile([HD, HD], CD, name="sw_sb")
        nc.sync.dma_start(sw_sb[:], sw_e[:, :])
        id_sb = consts.tile([HD, HD], CD, name="id_sb")
        nc.sync.dma_start(id_sb[:], id_e[:, :])
        mk_sb = consts.tile([HD, HD], F32, name="mk_sb")
        nc.sync.dma_start(mk_sb[:], mk_e[:, :])
        # resident weights: wq [d-tile, m] blocks, wk/wv per d-tile, wo per e-tile
        # (loads are emitted inside P1, interleaved with chunk-0 x tiles, so the
        # first projection matmul starts after a handful of DMAs)
        wq_sb = consts.tile([128, NDT * HPC * 128], CD, name="wq_sb")
        wk_sb = consts.tile([128, NDT * 128], CD, name="wk_sb")
        wv_sb = consts.tile([128, NDT * 128], CD, name="wv_sb")
        wo_sb = consts.tile([128, NET * 512], CD, name="wo_sb")
        qt_sb = qkv.tile([128, HPC * SEQ], CD, name="qt_sb")      # [hd, (h, seq)]
        kt_sb = qkv.tile([128, SEQ], CD, name="kt_sb")            # [hd, seq]
        v_sb = qkv.tile([128, (SEQ // 128) * (HD + 1)], CD, name="v_sb")  # [seqP,(t,129)]
        # ones column of v-hat (col 128 of each 129-block)
        nc.vector.memset(
            v_sb[:].rearrange("p (t c) -> p t c", c=HD + 1)[:, :, HD:HD + 1], 1.0
        )

        # ================= P1: projections + rope =================
        with tc.tile_pool(name="xin", bufs=44) as xin:
            for j in range(NCH):
                xts = []
                for d in range(NDT):
                    xt = xin.tile([128, 512], CD, tag="xin", name=f"x_{j}_{d}")
                    nc.sync.dma_start(xt[:], x_e[128 * d:128 * (d + 1), 512 * j:512 * (j + 1)])
                    xts.append(xt)
                    if j == 0:
                        nc.sync.dma_start(
                            wq_sb[:, 512 * d:512 * (d + 1)], wq_e[128 * d:128 * (d + 1), :]
                        )
                        nc.sync.dma_start(
                            wk_sb[:, 128 * d:128 * (d + 1)], wk_e[128 * d:128 * (d + 1), :]
                        )
                        nc.sync.dma_start(
                            wv_sb[:, 128 * d:128 * (d + 1)], wv_e[128 * d:128 * (d + 1), :]
                        )
                if j == 0:
                    nc.sync.dma_start(ch_sb[:], ch_e[:, :])
                    nc.sync.dma_start(sh_sb[:], sh_e[:, :])
                if j == min(1, NCH - 1):
                    for et in range(NET):
                        nc.sync.dma_start(
                            wo_sb[:, 512 * et:512 * (et + 1)],
                            wo_e[128 * et:128 * (et + 1), :],
                        )
                # q heads (m=0..3) and k (m=4): outputs in [hd, seq] layout
                def emit_rope(m, acc):
                    t_sb = rope.tile([128, 512], CD, tag="tsb", name=f"t_{j}_{m}")
                    nc.scalar.activation(t_sb[:], acc[:], AF.Copy)
                    ups = ps512.tile([128, 512], F32, tag="b512", name=f"u_{j}_{m}")
                    nc.tensor.matmul(ups[:], _mm(sw_sb[:]), _mm(t_sb[:]), start=True, stop=True)
                    m1 = rope.tile([128, 512], CD, tag="m1", name=f"m1_{j}_{m}")
                    nc.vector.tensor_tensor(
                        m1[:], t_sb[:], ch_sb[:, 512 * j:512 * (j + 1)], op=ALU.mult
                    )
                    if m < HPC:
                        dest = qt_sb[:, SEQ * m + 512 * j: SEQ * m + 512 * (j + 1)]
                    else:
                        dest = kt_sb[:, 512 * j:512 * (j + 1)]
                    nc.vector.tensor_tensor(
                        dest, ups[:], sh_sb[:, 512 * j:512 * (j + 1)], op=ALU.mult
                    )
                    nc.vector.tensor_add(dest, dest, m1[:])

                def wslice(m, d):
                    return (
                        wq_sb[:, 512 * d + 128 * m: 512 * d + 128 * (m + 1)]
                        if m < HPC
                        else wk_sb[:, 128 * d:128 * (d + 1)]
                    )

                # d-outer with multi-bank accumulation: consecutive matmuls hit
                # different psum banks (fill/drain overlap -> ~2x issue rate) and
                # chunk compute starts as soon as the first x tile lands
                qaccs = [
                    psbank.tile([128, 512], F32, tag="bank", name=f"dacc_{j}_{m}")
                    for m in range(HPC)
                ]
                kacc = ps512.tile([128, 512], F32, tag="b512", name=f"kacc_{j}")
                for d in range(NDT):
                    for m in range(HPC):
                        nc.tensor.matmul(
                            qaccs[m][:], _mm(wslice(m, d)), _mm(xts[d][:]),
                            start=(d == 0), stop=(d == NDT - 1),
                        )
                    nc.tensor.matmul(
                        kacc[:], _mm(wslice(HPC, d)), _mm(xts[d][:]),
                        start=(d == 0), stop=(d == NDT - 1),
                    )
                for m in range(HPC):
                    emit_rope(m, qaccs[m])
                emit_rope(HPC, kacc)
                # v in natural [seq, hd] layout
                vaccs = [
                    psbank.tile([128, 128], F32, tag="bank", name=f"vacc_{j}_{st}")
                    for st in range(4)
                ]
                for d in range(NDT):
                    for st in range(4):
                        nc.tensor.matmul(
                            vaccs[st][:],
                            _mm(xts[d][:, 128 * st:128 * (st + 1)]),
                            _mm(wv_sb[:, 128 * d:128 * (d + 1)]),
                            start=(d == 0),
                            stop=(d == NDT - 1),
                        )
                for st in range(4):
                    t = 4 * j + st
                    nc.scalar.activation(
                        v_sb[:, (HD + 1) * t:(HD + 1) * t + HD],
                        vaccs[st][:],
                        AF.Copy,
                    )

        # ================= P2/P3: attention + AG + out-proj =================
        with tc.tile_pool(name="wop", bufs=1) as wop, tc.tile_pool(name="agp", bufs=12) as agp:
            def emit_wo(j):
                wops = [
                    psbank.tile([128, 512], F32, tag="bank", name=f"wop_{j}_{sq}")
                    for sq in range(4)
                ]
                for et in range(NET):
                    agt = agp.tile([128, 512], CD, tag="agt", name=f"ag_{j}_{et}")
                    nc.sync.dma_start(agt[:], agout[j][128 * et:128 * (et + 1), :])
                    for sq in range(4):
                        nc.tensor.matmul(
                            wops[sq][:],
                            _mm(agt[:, 128 * sq:128 * (sq + 1)]),
                            _mm(wo_sb[:, 512 * et:512 * (et + 1)]),
                            start=(et == 0),
                            stop=(et == NET - 1),
                        )
                for sq in range(4):
                    oc = ocp.tile([128, 512], F32, tag="oc", name=f"oc_{j}_{sq}")
                    nc.scalar.activation(oc[:], wops[sq][:], AF.Copy)
                    nc.sync.dma_start(
                        out_e[512 * j + 128 * sq: 512 * j + 128 * (sq + 1), :], oc[:]
                    )

            for j in range(NCH):
                # ---- attention for seq chunk j, all 4 heads ----
                at_sb = atp.tile([128, HPC * 512], CD, tag="atT", name=f"atT{j}")
                for h in range(HPC):
                    qsl = qt_sb[:, SEQ * h + 512 * j: SEQ * h + 512 * (j + 1)]
                    aps = [
                        psbank.tile([128, HD + 1], F32, tag="bank", name=f"ap_{j}_{h}_{sq}")
                        for sq in range(4)
                    ]
                    for skt in range(4 * j + 4):
                        r = skt - 4 * j
                        lo = 128 * r if r > 0 else 0
                        stp = ps512.tile([128, 512], F32, tag="b512", name=f"st_{j}_{h}_{skt}")
                        nc.tensor.matmul(
                            stp[:, lo:512],
                            _mm(kt_sb[:, 128 * skt:128 * (skt + 1)]),
                            _mm(qsl[:, lo:512]),
                            start=True, stop=True,
                        )
                        E = epool.tile([128, 512], CD, tag="E", name=f"E_{j}_{h}_{skt}")
                        if r >= 0:
                            nc.vector.tensor_add(
                                stp[:, 128 * r:128 * (r + 1)],
                                stp[:, 128 * r:128 * (r + 1)],
                                mk_sb[:],
                            )
                            nc.scalar.activation(E[:, lo:512], stp[:, lo:512], AF.Exp)
                        else:
                            nc.scalar.activation(E[:], stp[:], AF.Exp)
                        for sq in range(max(0, r), 4):
                            nc.tensor.matmul(
                                aps[sq][:],
                                _mm(E[:, 128 * sq:128 * (sq + 1)]),
                                _mm(v_sb[:, (HD + 1) * skt:(HD + 1) * (skt + 1)]),
                                start=(skt == 0),
                                stop=(skt == 4 * j + sq),
                            )
                            if skt == 4 * j + sq:  # this sq-subtile is complete
                                inv = small.tile([128, 1], F32, tag="inv", name=f"i_{j}_{h}_{sq}")
                                nc.vector.reciprocal(inv[:], aps[sq][:, HD:HD + 1])
                                an = small.tile([128, 128], CD, tag="an", name=f"an_{j}_{h}_{sq}")
                                nc.vector.tensor_scalar_mul(
                                    an[:], aps[sq][:, 0:HD], inv[:]
                                )
                                trp = pstr.tile([128, 128], CD, tag="tr", name=f"tr_{j}_{h}_{sq}")
                                nc.tensor.transpose(trp[:], an[:], id_sb[:])
                                nc.vector.tensor_copy(
                                    at_sb[:, 512 * h + 128 * sq: 512 * h + 128 * (sq + 1)],
                                    trp[:],
                                )
                # ---- AllGather chunk j ----
                nc.sync.dma_start(
                    agin[j][:, :].rearrange("(h p) s -> p h s", h=HPC),
                    at_sb[:].rearrange("p (h s) -> p h s", h=HPC),
                )
                nc.gpsimd.collective_compute(
                    "AllGather",
                    ALU.bypass,
                    replica_groups=[list(range(NCORE))],
                    ins=[agin[j][:, :]],
                    outs=[agout[j][:, :]],
                )
                # ---- out projection, 2 chunks behind (software pipelined so
                # the PE stream never waits on an in-flight AllGather) ----
                if j > 1:
                    emit_wo(j - 2)
            for jj in range(max(0, NCH - 2), NCH):
                emit_wo(jj)


# ---------------- host side ----------------
_PERM = np.concatenate([np.arange(0, HD, 2), np.arange(1, HD, 2)])
_NC_CACHE = {}


def _get_nc():
    if "nc" not in _NC_CACHE:
        _NC_CACHE["nc"] = build()
    return _NC_CACHE["nc"]


def _prep_consts():
    freqs = 1.0 / (THETA ** (np.arange(HALF, dtype=np.float64) / HALF))
    ang = np.arange(SEQ, dtype=np.float64)[:, None] * freqs[None, :]
    cos = np.cos(ang).astype(np.float32)
    sin = np.sin(ang).astype(np.float32)
    CH = np.ascontiguousarray(np.concatenate([cos, cos], axis=1).T)
    SH = np.ascontiguousarray(np.concatenate([-sin, sin], axis=1).T)
    S_l = np.zeros((HD, HD), np.float32)
    for i in range(HD):
        S_l[(i + 64) % HD, i] = 1.0
    iden = np.eye(HD, dtype=np.float32)
    mask = np.where(
        np.arange(HD)[:, None] <= np.arange(HD)[None, :], 0.0, -1e30
    ).astype(np.float32)
    return CH, SH, S_l, iden, mask


def _cd(a):
    if COMPUTE == "bf16":
        import ml_dtypes
        return np.ascontiguousarray(a).astype(ml_dtypes.bfloat16)
    return np.ascontiguousarray(a, dtype=np.float32)


def kernel(x, wq, wk, wv, wo):
    x, wq, wk, wv, wo = (np.asarray(a, dtype=np.float32) for a in (x, wq, wk, wv, wo))
    nc = _get_nc()
    CH, SH, S_l, iden, mask = _prep_consts()
    xT = np.ascontiguousarray(x.T)
    wq_p = wq.reshape(NH, HD, DIM)[:, _PERM, :] * SCALE
    wk_p = wk.reshape(NKV, HD, DIM)[:, _PERM, :]
    xT_c = _cd(xT)
    CH_c, SH_c, S_c, id_c = _cd(CH), _cd(SH), _cd(S_l), _cd(iden)
    in_maps = []
    for c in range(NCORE):
        in_maps.append(
            {
                "x": xT_c,
                "wq": _cd(wq_p[HPC * c: HPC * (c + 1)].reshape(HPC * HD, DIM).T),
                "wk": _cd(wk_p[c].T),
                "wv": _cd(wv[HD * c: HD * (c + 1), :].T),
                "wo": _cd(wo[HPC * HD * c: HPC * HD * (c + 1), :].T),
                "cosz": CH_c,
                "sinz": SH_c,
                "swp": S_c,
                "iden": id_c,
                "mask": mask,
            }
        )
    res = run_bass_kernel_spmd(nc, in_maps, core_ids=list(range(NCORE)))
    out = np.concatenate([res.results[c]["out"] for c in range(NCORE)], axis=1)
    return np.ascontiguousarray(out, dtype=np.float32)
